# revision 16
# baseline (speedup 1.0000x reference)
"""Trainium2 Bass kernel for a dense transformer block (nn_Block_90185723281472).

Strategy (v2)
-------------
- Data-parallel over batch: B=8 -> 8 NeuronCores, one full block per core.
- Residual stream lives transposed in SBUF: [C(features, partitions), N(tokens)].
- Schedule is built for engine overlap + keeping the PE busy (HAM warm):
    LN1 -> {q(iti0), K, V} -> per token-tile iti: attention(iti) with
    q(iti+1)/normalize(iti-1)/proj(iti-1) as PE/DVE filler -> proj tail ->
    LN2 -> fc1/fc2 (hc-outer, gelu on Act, residual fused on DVE) -> channel
    attention -> out.
- Softmax: exp on Act engine, 2 j-chunks per instruction (strided PSUM read);
  padded keys contribute exp(0)*v=0 because both the k-pad and the V
  ones-column pad are zero, so no mask bias is needed.  Denominators are
  batched into a [12, w] tile and reciprocal'd in ONE DVE op per token tile;
  broadcast over 64 partitions on the (otherwise idle) GpSimd engine;
  in-place multiply on DVE.
- LN w/b are folded into qkv/fc1 weights host-side; LN apply is 2 fused ops
  split across GpSimd + DVE.  1/sqrt(var+eps) via one Act Rsqrt per phase.
- Weights are host-packed partition-major so every DMA descriptor is >=1.5KB.
"""

import os
import sys

for _p in ("/opt/trn_rl_repo", "/root/.axon_site/_ro/trn_rl_repo"):
    if os.path.isdir(_p) and _p not in sys.path:
        sys.path.insert(0, _p)

from contextlib import ExitStack

import ml_dtypes
import numpy as np

import concourse.bass as bass
import concourse.tile as tile
from concourse import bacc
from concourse import library_config
from concourse import mybir
from concourse.bass_utils import run_bass_kernel_spmd
from concourse.masks import make_identity

F32 = mybir.dt.float32
BF16 = mybir.dt.bfloat16
AF = mybir.ActivationFunctionType
ALU = mybir.AluOpType
AX = mybir.AxisListType

B = 8
C = 768
NT = 1025
NJP = 1152
NCC = C // 128          # 6
H, HD, NHP = 12, 64, 6
HID = 4 * C
NHC = HID // 128        # 24
NJC = NJP // 128        # 9
W = 342
ITS = [(0, 342), (342, 342), (684, 341)]
EPS = 1e-5
JG = [(0, 2), (2, 2), (4, 2), (6, 2), (8, 1)]   # exp j-chunk groups


def _build_program():
    nc = bacc.Bacc("TRN2", target_bir_lowering=False)

    xp = nc.dram_tensor("xp", [128, NCC, NT], F32, kind="ExternalInput").ap()
    wqkv = nc.dram_tensor("wqkv", [128, 18, NCC, 128], BF16,
                          kind="ExternalInput").ap()
    wproj = nc.dram_tensor("wproj", [128, NCC, NCC, 128], BF16,
                           kind="ExternalInput").ap()
    wfc1 = nc.dram_tensor("wfc1", [128, NHC, NCC, 128], BF16,
                          kind="ExternalInput").ap()
    wfc2 = nc.dram_tensor("wfc2", [128, NCC, NHC, 128], BF16,
                          kind="ExternalInput").ap()
    wca1 = nc.dram_tensor("wca1", [C, 192], F32, kind="ExternalInput").ap()
    wca2 = nc.dram_tensor("wca2", [192, C], F32, kind="ExternalInput").ap()
    bqkv = nc.dram_tensor("bqkv", [3 * C], F32, kind="ExternalInput").ap()
    bproj = nc.dram_tensor("bproj", [C], F32, kind="ExternalInput").ap()
    bfc1 = nc.dram_tensor("bfc1", [HID], F32, kind="ExternalInput").ap()
    bfc2 = nc.dram_tensor("bfc2", [C], F32, kind="ExternalInput").ap()
    outp = nc.dram_tensor("outp", [128, NCC, NT], F32,
                          kind="ExternalOutput").ap()

    with tile.TileContext(nc) as tc, ExitStack() as ctx:
        nc.gpsimd.load_library(library_config.attn)

        # ---------------- constants ----------------
        cpool = ctx.enter_context(tc.tile_pool(name="consts", bufs=1))
        ident = cpool.tile([128, 128], BF16, tag="ident", name="ident")
        make_identity(nc, ident)
        ones_c = cpool.tile([128, 1], BF16, tag="ones_c", name="ones_c")
        nc.vector.memset(ones_c, 1.0)
        eps_sb = cpool.tile([128, 1], F32, tag="eps_sb", name="eps_sb")
        nc.vector.memset(eps_sb, EPS)
        ones1 = cpool.tile([1, W], F32, tag="ones1", name="ones1")
        nc.vector.memset(ones1, 1.0)

        def load_pcvec(ap, nchunk, name):
            t = cpool.tile([128, nchunk], F32, tag=name, name=name)
            nc.sync.dma_start(t, ap.rearrange("(cc p) -> p cc", p=128))
            return t

        bqkv_sb = load_pcvec(bqkv, 18, "bqkv_sb")
        bproj_sb = load_pcvec(bproj, NCC, "bproj_sb")
        bfc1_sb = load_pcvec(bfc1, NHC, "bfc1_sb")
        bfc2_sb = load_pcvec(bfc2, NCC, "bfc2_sb")
        wca1_sb = cpool.tile([128, NCC, 192], F32, tag="wca1_sb",
                             name="wca1_sb")
        nc.sync.dma_start(wca1_sb, wca1.rearrange("(cc p) n -> p cc n", p=128))
        wca2a_sb = cpool.tile([128, C], F32, tag="wca2a", name="wca2a_sb")
        nc.sync.dma_start(wca2a_sb, wca2[0:128, :])
        wca2b_sb = cpool.tile([64, C], F32, tag="wca2b", name="wca2b_sb")
        nc.sync.dma_start(wca2b_sb, wca2[128:192, :])

        # shared pools
        acc = ctx.enter_context(tc.tile_pool(name="acc", bufs=2, space="PSUM"))
        wcol = ctx.enter_context(tc.tile_pool(name="wcol", bufs=2))
        lnbp = ctx.enter_context(tc.tile_pool(name="lnbp", bufs=2))
        lnt = ctx.enter_context(tc.tile_pool(name="lnt", bufs=3))
        lnstat = ctx.enter_context(tc.tile_pool(name="lnstat", bufs=1))
        dbp = ctx.enter_context(tc.tile_pool(name="dbp", bufs=2))

        # ---------------- persistent SBUF (late-dying first) -------------
        x1T, free_x1T = tc.tile([128, NCC, NT], F32, name="x1T")
        oT, free_oT = tc.tile([128, NCC, NT], BF16, name="oT")
        h2T, free_h2T = tc.tile([128, NCC, NT], BF16, name="h2T")
        kT, free_kT = tc.tile([128, NCC, NJP], BF16, name="kT")
        qT, free_qT = tc.tile([128, NCC, NT], BF16, name="qT")
        vnat, free_vnat = tc.tile([128, NJC, H, HD + 1], BF16, name="vnat")
        wproj_sb, free_wproj = tc.tile([128, NCC, NCC, 128], BF16,
                                       name="wproj_sb")
        xsb, free_xsb = tc.tile([128, NCC, NT], F32, name="xsb")
        wq_sb, free_wq = tc.tile([128, NCC, NCC, 128], BF16, name="wq_sb")
        hT, free_hT = tc.tile([128, NCC, NT], BF16, name="hT")
        vT, free_vT = tc.tile([128, NCC, NT], BF16, name="vT")

        for cc in range(NCC):
            nc.sync.dma_start(xsb[:, cc, :], xp[:, cc, :])
        nc.sync.dma_start(wq_sb, wqkv[:, 0:NCC, :, :])
        nc.sync.dma_start(wproj_sb, wproj)
        nc.vector.memset(kT[:, :, NT:NJP], 0.0)
        nc.vector.memset(vnat, 0.0)
        for jb in range(NJC - 1):
            nc.vector.memset(vnat[:, jb, :, HD:HD + 1], 1.0)
        nc.vector.memset(vnat[0:1, NJC - 1, :, HD:HD + 1], 1.0)


        # ---------------- LayerNorm helpers ----------------
        # stats live on partition 0 as [1, 3(iti), W] rows
        def ln_stats(src3, tag):
            mstat = lnstat.tile([1, 3, W], F32, tag="lnm", name=f"m_{tag}")
            sstat = lnstat.tile([1, 3, W], F32, tag="lns", name=f"s_{tag}")

            def stats_iti(iti, lsp):
                io, w = ITS[iti]
                mu_ps = lsp.tile([1, W], F32, tag="lnps", name="mu_ps")
                s2_ps = lsp.tile([1, W], F32, tag="lnps", name="s2_ps")
                for cc in range(NCC):
                    xb = lnt.tile([128, W], BF16, tag="xb", name="xb")
                    if cc % 2 == 0:
                        nc.scalar.activation(xb[:, :w], src3(cc, io, w),
                                             AF.Identity)
                    else:
                        nc.vector.tensor_copy(xb[:, :w], src3(cc, io, w))
                    sq = lnt.tile([128, W], BF16, tag="sq", name="sq")
                    nc.vector.tensor_mul(sq[:, :w], xb[:, :w], xb[:, :w])
                    nc.tensor.matmul(mu_ps[:, :w], ones_c, xb[:, :w],
                                     start=(cc == 0), stop=(cc == NCC - 1))
                    nc.tensor.matmul(s2_ps[:, :w], ones_c, sq[:, :w],
                                     start=(cc == 0), stop=(cc == NCC - 1))
                nc.vector.tensor_scalar_mul(mstat[0:1, iti, :w],
                                            mu_ps[:, :w], 1.0 / C)
                nc.vector.tensor_scalar_mul(sstat[0:1, iti, :w],
                                            s2_ps[:, :w], 1.0 / C)
            return mstat, sstat, stats_iti

        def ln_statproc(mstat, sstat, tag):
            # rs = 1/sqrt(var+eps) = exp(-0.5*ln(var+eps)); ln/exp share the
            # same Act table as the softmax exp -> no table thrash.
            musq = lnstat.tile([1, 3, W], F32, tag="lnmq", name=f"mq_{tag}")
            nc.vector.tensor_mul(musq, mstat, mstat)
            # var -> sstat (in place), ln(var+eps) -> musq (reuse)
            nc.vector.tensor_sub(sstat, sstat, musq)
            nc.scalar.activation(musq, sstat, AF.Ln, bias=eps_sb[0:1, :])
            rstat = lnstat.tile([1, 3, W], F32, tag="lnr", name=f"r_{tag}")
            nc.scalar.activation(rstat, musq, AF.Exp, scale=-0.5)
            return rstat

        def ln_bcast(mstat, rstat, iti, w):
            bmu = lnbp.tile([128, W], F32, tag="bmu", name="bmu")
            nc.gpsimd.partition_broadcast(bmu[:, :w], mstat[0:1, iti, :w])
            brs = lnbp.tile([128, W], F32, tag="brs", name="brs")
            nc.gpsimd.partition_broadcast(brs[:, :w], rstat[0:1, iti, :w])
            return bmu, brs

        def ln_apply_cc(src, dst, bmu, brs, w):
            t1 = lnt.tile([128, W], F32, tag="t1", name="t1")
            nc.vector.tensor_tensor(t1[:, :w], src, bmu[:, :w],
                                    op=ALU.subtract)
            nc.vector.tensor_tensor(dst, t1[:, :w], brs[:, :w], op=ALU.mult)

        # ================ Phase 1: LN1 ================
        m1, s1, stats1 = ln_stats(lambda cc, io, w: xsb[:, cc, io:io + w],
                                  "ln1")
        with tc.tile_pool(name="lnps1", bufs=2, space="PSUM") as lsp1:
            for iti in range(3):
                stats1(iti, lsp1)
        r1 = ln_statproc(m1, s1, "ln1")
        for iti in range(3):
            io, w = ITS[iti]
            bmu, brs = ln_bcast(m1, r1, iti, w)
            for cc in range(NCC):
                ln_apply_cc(xsb[:, cc, io:io + w], hT[:, cc, io:io + w],
                            bmu, brs, w)

        # ================ Phase 2: q(iti0), K, V ================
        def qkv_mm(wt_cc, oc, iti):
            """one qkv output unit given weight slices wt_cc[cc] -> [128,128]"""
            io, w = ITS[iti]
            ps = acc.tile([128, W], F32, tag="acc", name="qkv_ps")
            for cc in range(NCC):
                nc.tensor.matmul(ps[:, :w], wt_cc(cc), hT[:, cc, io:io + w],
                                 start=(cc == 0), stop=(cc == NCC - 1))
            bias = bqkv_sb[:, oc:oc + 1]
            if oc < NCC:
                nc.vector.tensor_scalar_add(qT[:, oc, io:io + w],
                                            ps[:, :w], bias)
            elif oc < 2 * NCC:
                nc.vector.tensor_scalar_add(kT[:, oc - NCC, io:io + w],
                                            ps[:, :w], bias)
            else:
                nc.vector.tensor_scalar_add(vT[:, oc - 2 * NCC, io:io + w],
                                            ps[:, :w], bias)

        def q_unit(oc, iti):
            qkv_mm(lambda cc: wq_sb[:, oc, cc, :], oc, iti)

        def kv_chunk(oc, acc3):
            wt = wcol.tile([128, NCC, 128], BF16, tag="wcol", name="wt_kv")
            nc.sync.dma_start(wt, wqkv[:, oc, :, :])
            ps3 = acc3.tile([128, 3, 512], F32, tag="acc3", name="kv_ps3")
            for cc in range(NCC):
                for iti in range(3):
                    io, w = ITS[iti]
                    nc.tensor.matmul(ps3[:, iti, :w], wt[:, cc, :],
                                     hT[:, cc, io:io + w],
                                     start=(cc == 0), stop=(cc == NCC - 1))
            bias = bqkv_sb[:, oc:oc + 1]
            for iti in range(3):
                io, w = ITS[iti]
                if oc < 2 * NCC:
                    nc.vector.tensor_scalar_add(kT[:, oc - NCC, io:io + w],
                                                ps3[:, iti, :w], bias)
                else:
                    nc.vector.tensor_scalar_add(vT[:, oc - 2 * NCC, io:io + w],
                                                ps3[:, iti, :w], bias)

        with (
            tc.tile_pool(name="vtrp", bufs=2, space="PSUM") as vtrp,
            tc.tile_pool(name="acc3a", bufs=1, space="PSUM") as acc3a,
        ):
            def v_transpose(vc, jb):
                ncol = 128 if jb < NJC - 1 else 1
                tp = vtrp.tile([128, 128], BF16, tag="tr", name="vtr")
                nc.tensor.transpose(tp[0:ncol, :],
                                    vT[:, vc, jb * 128:jb * 128 + ncol],
                                    ident)
                nc.vector.tensor_copy(vnat[0:ncol, jb, 2 * vc, 0:HD],
                                      tp[0:ncol, 0:64])
                nc.vector.tensor_copy(vnat[0:ncol, jb, 2 * vc + 1, 0:HD],
                                      tp[0:ncol, 64:128])

            for oc in range(NCC):
                q_unit(oc, 0)
            for oc in range(NCC, 2 * NCC):
                kv_chunk(oc, acc3a)
            for oc in range(2 * NCC, 3 * NCC):
                kv_chunk(oc, acc3a)
                vc = oc - 2 * NCC
                for jb in range(NJC):
                    v_transpose(vc, jb)
        free_vT()

        # ================ Phase 3: attention per iti ================
        attn_ctx = ExitStack()
        spool = attn_ctx.enter_context(tc.tile_pool(name="spool", bufs=2,
                                                    space="PSUM"))
        opool = attn_ctx.enter_context(tc.tile_pool(name="opool", bufs=2,
                                                    space="PSUM"))
        ptp = attn_ctx.enter_context(tc.tile_pool(name="ptp", bufs=4))
        scale = float(HD) ** -0.5

        def attn_unit(hp, h2, iti):
            io, w = ITS[iti]
            pb = 64 * h2
            h = 2 * hp + h2
            o_ps = opool.tile([HD + 1, 512], F32, tag="o", name="o_ps")
            pts = []
            for j0, gn in JG:
                s = spool.tile([128, 2, 512], F32, tag="s", name="s_ps")
                for k in range(gn):
                    jc = j0 + k
                    nc.tensor.matmul(
                        s[:, k, :w],
                        kT[pb:pb + 64, hp, jc * 128:(jc + 1) * 128],
                        qT[pb:pb + 64, hp, io:io + w])
                pt = ptp.tile([128, 2, W], BF16, tag="pt", name="pt")
                nc.scalar.activation(pt[:, 0:gn, :w], s[:, 0:gn, :w],
                                     AF.Exp, scale=scale)
                pts.append((pt, j0, gn))
            for pt, j0, gn in pts:
                for k in range(gn):
                    jc = j0 + k
                    nc.tensor.matmul(o_ps[:, :w], vnat[:, jc, h, :],
                                     pt[:, k, :w],
                                     start=(jc == 0), stop=(jc == NJC - 1))
            dn = dbp.tile([1, W], F32, tag="dn", name="dn")
            nc.vector.tensor_tensor(dn[0:1, :w], o_ps[HD:HD + 1, :w],
                                    ones1[:, :w], op=ALU.mult)
            rcp = dbp.tile([1, W], F32, tag="rcp", name="rcp")
            nc.vector.reciprocal_approx_fast(rcp[0:1, :w], dn[0:1, :w])
            db = dbp.tile([64, W], F32, tag="db", name="db")
            nc.gpsimd.partition_broadcast(db[:, :w], rcp[0:1, :w])
            nc.vector.tensor_tensor(oT[pb:pb + 64, hp, io:io + w],
                                    o_ps[0:HD, :w], db[:, :w], op=ALU.mult)

        def proj_unit(oc, iti):
            io, w = ITS[iti]
            ps = acc.tile([128, W], F32, tag="acc", name="proj_ps")
            for cc in range(NCC):
                nc.tensor.matmul(ps[:, :w], wproj_sb[:, oc, cc, :],
                                 oT[:, cc, io:io + w],
                                 start=(cc == 0), stop=(cc == NCC - 1))
            nc.vector.scalar_tensor_tensor(x1T[:, oc, io:io + w], ps[:, :w],
                                           bproj_sb[:, oc:oc + 1],
                                           xsb[:, oc, io:io + w],
                                           op0=ALU.add, op1=ALU.add)

        def fillers(iti):
            fs = []
            if iti + 1 < 3:
                for oc in range(NCC):
                    fs.append(lambda oc=oc, it=iti + 1: q_unit(oc, it))
            if iti > 0:
                pit = iti - 1
                for oc in range(NCC):
                    fs.append(lambda oc=oc, pit=pit: proj_unit(oc, pit))
            return fs

        for iti in range(3):
            fills = fillers(iti)
            fi = 0
            units = [(hp, h2) for hp in range(NHP) for h2 in range(2)]
            for ui, (hp, h2) in enumerate(units):
                attn_unit(hp, h2, iti)
                want = (ui + 1) * len(fills) // len(units)
                while fi < want:
                    fills[fi]()
                    fi += 1
            while fi < len(fills):
                fills[fi]()
                fi += 1

        attn_ctx.close()
        free_hT()
        free_wq()
        for oc in range(NCC):
            proj_unit(oc, 2)
        free_xsb()
        free_wproj()

        # ================ Phase 4: LN2 ================
        m2, s2, stats2 = ln_stats(lambda cc, io, w: x1T[:, cc, io:io + w],
                                  "ln2")
        with tc.tile_pool(name="lnps2", bufs=2, space="PSUM") as lsp2:
            for iti in range(3):
                stats2(iti, lsp2)
        r2 = ln_statproc(m2, s2, "ln2")
        for iti in range(3):
            io, w = ITS[iti]
            bmu, brs = ln_bcast(m2, r2, iti, w)
            for cc in range(NCC):
                ln_apply_cc(x1T[:, cc, io:io + w], h2T[:, cc, io:io + w],
                            bmu, brs, w)
        for _f in (free_vnat, free_qT, free_kT):
            _f()

        # ================ Phase 5: MLP (hc-outer, weights loaded once) ----
        mlpT, free_mlpT = tc.tile([128, NHC, NT], BF16, name="mlpT")
        camx = cpool.tile([128, NCC, 3], F32, tag="camx", name="camx")
        cavg = cpool.tile([128, NCC, 3], F32, tag="cavg", name="cavg")

        with (
            tc.tile_pool(name="acc3b", bufs=2, space="PSUM") as acc3b,
            tc.tile_pool(name="w2p", bufs=2) as w2p,
        ):
            for hc in range(NHC):
                wt = wcol.tile([128, NCC, 128], BF16, tag="wcol",
                               name="wt_fc1")
                nc.sync.dma_start(wt, wfc1[:, hc, :, :])
                ps3 = acc3b.tile([128, 3, 512], F32, tag="acc3",
                                 name="fc1_ps3")
                for cc in range(NCC):
                    for iti in range(3):
                        io, w = ITS[iti]
                        nc.tensor.matmul(ps3[:, iti, :w], wt[:, cc, :],
                                         h2T[:, cc, io:io + w],
                                         start=(cc == 0),
                                         stop=(cc == NCC - 1))
                for iti in range(3):
                    io, w = ITS[iti]
                    nc.scalar.activation(mlpT[:, hc, io:io + w],
                                         ps3[:, iti, :w], AF.Gelu,
                                         bias=bfc1_sb[:, hc:hc + 1])
            for oc in range(NCC):
                wt2 = w2p.tile([128, NHC, 128], BF16, tag="w2", name="wt_fc2")
                nc.sync.dma_start(wt2, wfc2[:, oc, :, :])
                ps3 = acc3b.tile([128, 3, 512], F32, tag="acc3",
                                 name="fc2_ps3")
                for hc in range(NHC):
                    for iti in range(3):
                        io, w = ITS[iti]
                        nc.tensor.matmul(ps3[:, iti, :w], wt2[:, hc, :],
                                         mlpT[:, hc, io:io + w],
                                         start=(hc == 0),
                                         stop=(hc == NHC - 1))
                for iti in range(3):
                    io, w = ITS[iti]
                    nc.vector.scalar_tensor_tensor(x1T[:, oc, io:io + w],
                                                   ps3[:, iti, :w],
                                                   bfc2_sb[:, oc:oc + 1],
                                                   x1T[:, oc, io:io + w],
                                                   op0=ALU.add, op1=ALU.add)
                    jo = max(io, 1)
                    nc.vector.tensor_reduce(camx[:, oc, iti:iti + 1],
                                            x1T[:, oc, jo:io + w], axis=AX.X,
                                            op=ALU.max)
                    nc.vector.reduce_sum(cavg[:, oc, iti:iti + 1],
                                         x1T[:, oc, jo:io + w], axis=AX.X)

        # ================ Phase 6: channel attention ================
        with (
            tc.tile_pool(name="cap", bufs=1) as cap,
            tc.tile_pool(name="caps", bufs=2, space="PSUM") as caps,
        ):
            mx = cap.tile([128, NCC], F32, tag="mx", name="ca_mx")
            av = cap.tile([128, NCC], F32, tag="av", name="ca_av")
            for oc in range(NCC):
                nc.vector.tensor_reduce(mx[:, oc:oc + 1], camx[:, oc, :],
                                        axis=AX.X, op=ALU.max)
                nc.vector.reduce_sum(av[:, oc:oc + 1], cavg[:, oc, :],
                                     axis=AX.X)
            nc.vector.tensor_scalar_mul(av, av, 1.0 / (NT - 1))
            relus = []
            for bi, pool_t in enumerate((mx, av)):
                ga = caps.tile([128, 1], F32, tag="g1a", name="ca_ga")
                gb = caps.tile([64, 1], F32, tag="g1b", name="ca_gb")
                for cc in range(NCC):
                    nc.tensor.matmul(ga, wca1_sb[:, cc, 0:128],
                                     pool_t[:, cc:cc + 1],
                                     start=(cc == 0), stop=(cc == NCC - 1))
                    nc.tensor.matmul(gb, wca1_sb[:, cc, 128:192],
                                     pool_t[:, cc:cc + 1],
                                     start=(cc == 0), stop=(cc == NCC - 1))
                ra = cap.tile([128, 1], F32, tag=f"ra{bi}", name="ca_ra")
                nc.vector.tensor_relu(ra, ga)
                rb = cap.tile([64, 1], F32, tag=f"rb{bi}", name="ca_rb")
                nc.vector.tensor_relu(rb, gb)
                relus.append((ra, rb))
            for oc in range(NCC):
                gt = caps.tile([128, 1], F32, tag="gt", name="ca_gt")
                k = 0
                for ra, rb in relus:
                    nc.tensor.matmul(gt, wca2a_sb[:, oc * 128:(oc + 1) * 128],
                                     ra, start=(k == 0), stop=False)
                    k += 1
                    nc.tensor.matmul(gt, wca2b_sb[:, oc * 128:(oc + 1) * 128],
                                     rb, start=False, stop=(k == 3))
                    k += 1
                gs = cap.tile([128, 1], F32, tag=f"gs{oc}", name="ca_gs")
                nc.scalar.activation(gs, gt, AF.Sigmoid)
                nc.vector.tensor_scalar_add(gs, gs, 1.0)
                if oc % 2 == 0:
                    nc.vector.tensor_scalar_mul(x1T[:, oc, 1:NT],
                                                x1T[:, oc, 1:NT], gs)
                else:
                    nc.scalar.activation(x1T[:, oc, 1:NT], x1T[:, oc, 1:NT],
                                         AF.Identity, scale=gs)
                nc.sync.dma_start(outp[:, oc, :], x1T[:, oc, :])

        for _f in (free_mlpT, free_h2T, free_oT,
                   free_x1T):
            _f()

    nc.compile()
    return nc


_CACHE = {}


def _get_program():
    if "nc" not in _CACHE:
        _CACHE["nc"] = _build_program()
    return _CACHE["nc"]


def _make_in_maps(inputs):
    bf = ml_dtypes.bfloat16
    f32 = np.float32

    def as_np(a, dt=f32):
        return np.ascontiguousarray(np.asarray(a, dtype=f32).astype(dt))

    def pack_w(w, nout, nin=NCC):
        # [nin*128, nout*128] -> [128, nout, nin, 128] partition-major
        a = np.asarray(w, dtype=f32)
        a = a.reshape(nin, 128, nout, 128).transpose(1, 2, 0, 3)
        return np.ascontiguousarray(a.astype(bf))

    ln1w = np.asarray(inputs["ln1_w"], dtype=f32)
    ln1b = np.asarray(inputs["ln1_b"], dtype=f32)
    ln2w = np.asarray(inputs["ln2_w"], dtype=f32)
    ln2b = np.asarray(inputs["ln2_b"], dtype=f32)
    qkv_w = np.asarray(inputs["qkv_w"], dtype=f32)
    fc1_w = np.asarray(inputs["fc1_w"], dtype=f32)

    base = {
        "wqkv": pack_w(ln1w[:, None] * qkv_w, 18),
        "wproj": pack_w(inputs["proj_w"], NCC),
        "wfc1": pack_w(ln2w[:, None] * fc1_w, NHC),
        "wfc2": pack_w(inputs["fc2_w"], NCC, nin=NHC),
        "wca1": as_np(inputs["ca1_w"]),
        "wca2": as_np(inputs["ca2_w"]),
        "bqkv": np.ascontiguousarray(ln1b @ qkv_w).astype(f32),
        "bproj": as_np(inputs["proj_b"]),
        "bfc1": np.ascontiguousarray(
            np.asarray(inputs["fc1_b"], dtype=f32) + ln2b @ fc1_w),
        "bfc2": as_np(inputs["fc2_b"]),
    }
    x = np.asarray(inputs["x"], dtype=f32)
    in_maps = []
    for b in range(B):
        m = dict(base)
        xt = x[b].T.reshape(NCC, 128, NT).transpose(1, 0, 2)
        m["xp"] = np.ascontiguousarray(xt)
        in_maps.append(m)
    return in_maps


def _unpack_out(res):
    out = np.empty((B, NT, C), dtype=np.float32)
    for b in range(B):
        o = np.asarray(res.results[b]["outp"])  # [128, NCC, NT]
        out[b] = o.transpose(1, 0, 2).reshape(C, NT).T
    return out


def kernel(**inputs) -> np.ndarray:
    nc = _get_program()
    in_maps = _make_in_maps(inputs)
    res = run_bass_kernel_spmd(nc, in_maps, list(range(B)))
    return _unpack_out(res)


if __name__ == "__main__":
    nc = _get_program()
    n_inst = sum(len(bb.instructions) for bb in nc.main_func.blocks)
    print(f"program built: {n_inst} instructions")
